# revision 27
# baseline (speedup 1.0000x reference)
"""GCN (3-layer GCNConv + global mean pool) on 8 Trainium2 NeuronCores.

Math: with S = adjacency+self-loops and D = diag(1/sqrt(deg)),
    conv(h) = relu(D S D h W + b)
and the diagonal scalings commute with the dense W, so each layer is an
UNWEIGHTED gather-sum of pre-scaled rows plus a dense matmul.  The final
conv + mean-pool collapse into a dense host-precomputed pooling matrix.

Sharding: nodes dst-partitioned across 8 cores; layer 0 is recomputed on
each core's halo so cores never communicate.  Layer-0 slots are ordered
[own nodes | halo-only nodes] so each core's own h1 tiles stay resident
in SBUF: the layer-1 self-loop contribution is two identity matmuls per
tile instead of 12.5K gather descriptors.  h1 is stored fp8 with 4
consecutive rows packed per partition (1 KiB contiguous runs -> full DMA
write bandwidth).  Layer 1 gathers h1 rows with the GPSIMD dma_gather
(int16 indices -> two <=32257-row windows) and aggregates pairs of
gathered columns with fp8 DoubleRow identity matmuls (2 messages per PE
instruction).  The gather is split into a window-0 phase that overlaps
the layer-0 tail (partial sums parked in a bf16 SBUF stash) and a
window-1 phase issued after the last h1 write, so the in-order GPSIMD
queue never stalls early gathers behind late-window dependencies.
All index arithmetic happens on the host.
"""

import hashlib

import numpy as np
import ml_dtypes

P = 128
NCORES = 8
WSTR = 32768      # physical window stride (rows); row w*WSTR is all-zero
WIN_STAGES = 63   # stages (4 tiles = 512 rows) per gather window
GC0 = 32          # layer-0 table pair-columns per DMA chunk
GCOLS = 64        # layer-1 gather column budget per tile-group (window 0)
GCOLS1 = 32       # smaller window-1 groups for a shorter phase-B tail
CCAP = 32         # max columns per dma_gather call

BF16 = ml_dtypes.bfloat16
F8 = ml_dtypes.float8_e4m3    # TRN FP8_EXP4-compatible for |v| <= 240

# power-of-2 pre-scales keeping fp8 operands out of the subnormal range;
# compensated exactly by the fp32 activation scales and the pooling matrix
S_T, S_W0, S_H1, S_H2 = 32.0, 32.0, 32.0, 16.0
ACT0_SCALE = S_H1 / (S_T * S_W0)
ACT1_SCALE = S_H2 / S_H1


def _f8(a):
    return np.clip(np.ascontiguousarray(a), -240, 240).astype(F8)


def _win_of_stage(s):
    return s // WIN_STAGES


def _loc_of_slot(s):
    """h1h row (1-based within window; 0 = window zero row) of slot s.
    Stage rows are packed partition-major: partition p holds its 4 tiles'
    rows consecutively (1 KiB fp8 runs)."""
    t = s // P
    p = s % P
    st = t // 4
    j = t % 4
    w = _win_of_stage(st)
    return w, 1 + (st - w * WIN_STAGES) * 512 + p * 4 + j


# ---------------------------------------------------------------------------
# Shared schedule derivation (host + builder + emulator all use this)
# ---------------------------------------------------------------------------

def _schedule_w(Dw, budget=GCOLS):
    """Per-window schedule.  Dw: [T1] column counts for this window.
    Returns (groups, calls, colbase, Ctot).  Groups start/end on even
    tiles (device processes tile pairs).  calls are (ncols, coloff)
    chunks <= CCAP that never split a DoubleRow pair (pairs are the
    even-offset column pairs within each tile's run)."""
    T = len(Dw)
    groups = []
    t = 0
    while t < T:
        tot = Dw[t] + (Dw[t + 1] if t + 1 < T else 0)
        t1 = min(t + 2, T)
        while t1 + 1 < T and tot + Dw[t1] + Dw[t1 + 1] <= budget:
            tot += Dw[t1] + Dw[t1 + 1]
            t1 += 2
        groups.append((t, t1))
        t = t1
    colbase = [0] * (T + 1)
    for t in range(T):
        colbase[t + 1] = colbase[t] + Dw[t]
    calls = []
    for (t0, t1) in groups:
        g0, g1 = colbase[t0], colbase[t1]
        c = g0
        while c < g1:
            n = min(CCAP, g1 - c)
            # find the run containing the break point c+n; back off one
            # column if the break splits a pair inside that run
            e = c + n
            if e < g1:
                # pairs start at even offsets within each tile's run; a
                # break at an odd offset would split a DoubleRow pair
                tt = t0
                while colbase[tt + 1] <= e - 1:
                    tt += 1
                if (e - colbase[tt]) % 2 == 1:
                    n -= 1
            calls.append((n, c))
            c += n
    return groups, calls, colbase, colbase[T]


def _schedule0(D0P):
    """Layer-0 chunking: greedy tile groups with <= GC0 pair-columns each.
    Returns (groups, colbase) with colbase[t] the global pair-col offset."""
    T = len(D0P)
    groups = []
    t = 0
    while t < T:
        tot = D0P[t]
        t1 = t + 1
        while t1 < T and tot + D0P[t1] <= GC0:
            tot += D0P[t1]
            t1 += 1
        groups.append((t, t1))
        t = t1
    colbase = [0] * (T + 1)
    for t in range(T):
        colbase[t + 1] = colbase[t] + D0P[t]
    return groups, colbase


def _call_of_col(calls):
    """Map window-local column -> (call_id, local_col)."""
    m = {}
    for ci, (n, off) in enumerate(calls):
        for j in range(n):
            m[off + j] = (ci, j)
    return m


# ---------------------------------------------------------------------------
# Host preprocessing
# ---------------------------------------------------------------------------

def _edge_expand(nodes, degi, s_sorted, indptr):
    """Expand in-edge lists (self-loop last) for `nodes`.
    Returns (rep, jj, srcs): for each edge, owning node position in
    `nodes`, edge rank, and src global id."""
    dg = degi[nodes]
    tot = int(dg.sum())
    rep = np.repeat(np.arange(len(nodes)), dg)
    jj = np.arange(tot) - np.repeat(np.cumsum(dg) - dg, dg)
    g = nodes[rep]
    is_self = jj == (dg[rep] - 1)
    ei = np.minimum(indptr[g] + jj, len(s_sorted) - 1)
    srcs = np.where(is_self, g, s_sorted[ei])
    return rep, jj, srcs


def _edge_expand_noself(nodes, degi, s_sorted, indptr):
    """Real in-edges only (no self-loop)."""
    dg = degi[nodes] - 1
    rep = np.repeat(np.arange(len(nodes)), dg)
    jj = np.arange(int(dg.sum())) - np.repeat(np.cumsum(dg) - dg, dg)
    srcs = s_sorted[indptr[nodes[rep]] + jj]
    return rep, jj, srcs


def _preprocess(x, edge_index, batch, num_graphs, W0, b0, W1, b1, W2, b2):
    x = np.asarray(x, np.float32)
    N, IN = x.shape
    HID = W0.shape[1]
    G = int(num_graphs)
    SH = N // NCORES
    src = np.asarray(edge_index[0], np.int64)
    dst = np.asarray(edge_index[1], np.int64)
    batch = np.asarray(batch, np.int64)

    degi = np.bincount(dst, minlength=N) + 1          # + self-loop
    dis = (1.0 / np.sqrt(degi.astype(np.float64))).astype(np.float32)
    invd = np.sqrt(degi.astype(np.float64)).astype(np.float32)

    order = np.argsort(dst, kind="stable")
    s_sorted = src[order]
    indptr = np.searchsorted(dst, np.arange(N + 1), sorter=order)

    xhat = (x * dis[:, None]).astype(np.float32)
    dis2 = (dis * dis).astype(np.float32)

    # dense pooling matrix Mhat = diag(1/cnt) @ S_pool @ A_norm  [G, N]
    cnt = np.bincount(batch, minlength=G).astype(np.float64)
    cntc = np.maximum(cnt, 1.0)
    bd = batch[dst]
    w_ = dis[dst].astype(np.float64) * dis[src] / cntc[bd]
    M = np.bincount(bd * N + src, weights=w_, minlength=G * N)
    w2_ = dis.astype(np.float64) ** 2 / cntc[batch]
    M += np.bincount(batch * N + np.arange(N), weights=w2_, minlength=G * N)
    Mhat = M.reshape(G, N).astype(np.float32)

    T1 = -(-(-(-SH // P)) // 4) * 4

    owns, halos_only = [], []
    for c in range(NCORES):
        own = np.arange(c * SH, (c + 1) * SH)
        h = np.unique(s_sorted[indptr[c * SH]:indptr[(c + 1) * SH]])
        owns.append(own)
        halos_only.append(np.setdiff1d(h, own, assume_unique=True))

    TH = max(-(-len(h) // P) for h in halos_only)
    T0 = -(-(T1 + TH) // 4) * 4
    TH = T0 - T1
    S0 = T0 // 4
    NW = -(-S0 // WIN_STAGES)
    assert NW <= 2, "layer-1 source exceeds two windows"

    # halo-only slots: degree-sorted (pads at end)
    slot_h = []
    for c in range(NCORES):
        ho = halos_only[c]
        o = np.argsort(-degi[ho], kind="stable")
        sn = np.full(TH * P, -1, np.int64)
        sn[:len(ho)] = ho[o]
        slot_h.append(sn)

    # per-own-node window count vectors (real in-edges; own srcs are
    # always in window 0 since own tiles come first)
    wvecs = []
    for c in range(NCORES):
        own = owns[c]
        pos_h = np.full(N, -1, np.int64)
        v = slot_h[c] >= 0
        pos_h[slot_h[c][v]] = np.nonzero(v)[0] + T1 * P
        srcs = s_sorted[indptr[c * SH]:indptr[(c + 1) * SH]]
        own_rep = np.repeat(np.arange(SH), degi[own] - 1)
        is_own_src = (srcs >= c * SH) & (srcs < (c + 1) * SH)
        slotpos = np.where(is_own_src, 0, pos_h[srcs])
        w_e = (slotpos // P // 4) // WIN_STAGES if NW == 2 else \
            np.zeros(len(srcs), np.int64)
        wv = np.zeros((SH, NW), np.int64)
        np.add.at(wv, (own_rep, w_e), 1)
        wvecs.append(wv)

    # own slots: primary sort by degree (tight shared D0 profile), then by
    # window vector (tight shared D1 profile); pads at end
    slot_o = []
    for c in range(NCORES):
        own, wv = owns[c], wvecs[c]
        keys = tuple(wv[:, i] for i in range(NW - 1, -1, -1)) + \
            (-degi[own],)
        o = np.lexsort(keys)
        sn = np.full(T1 * P, -1, np.int64)
        sn[:SH] = own[o]
        slot_o.append(sn)

    slot0 = [np.concatenate([slot_o[c], slot_h[c]]) for c in range(NCORES)]

    degmat = np.zeros((NCORES, T0 * P), np.int64)
    for c in range(NCORES):
        v = slot0[c] >= 0
        degmat[c][v] = degi[slot0[c][v]]
    D0 = degmat.reshape(NCORES, T0, P).max(axis=(0, 2))
    D0Pt = tuple(-(-int(v) // 2) for v in D0)
    groups0, colbase0 = _schedule0(D0Pt)
    C0P = colbase0[T0]

    # layer-1 per-window profiles
    D1s = []
    for c in range(NCORES):
        wv_full = np.zeros((N, NW), np.int64)
        wv_full[owns[c]] = wvecs[c]
        vec = np.zeros((T1 * P, NW), np.int64)
        v = slot_o[c] >= 0
        vec[v] = wv_full[slot_o[c][v]]
        D1s.append(vec.reshape(T1, P, NW).max(axis=1))
    D1 = np.max(D1s, axis=0)          # [T1, NW]
    D1t = tuple(tuple(int(v) for v in row) for row in D1)
    scheds = [_schedule_w(tuple(int(D1[t][w]) for t in range(T1)),
                          GCOLS if w == 0 else GCOLS1)
              for w in range(NW)]
    C1w = [s[3] for s in scheds]
    Ctot = sum(C1w)
    wbase = [0, C1w[0]][:NW]

    cb0 = np.asarray(colbase0[:T0], np.int64)
    cores = []
    for c in range(NCORES):
        s0 = slot0[c]
        v0 = s0 >= 0

        # layer-0 fused message table: transposed pair layout, fp8.
        X0 = np.zeros((C0P * 2, P, IN), np.float32)
        k = np.nonzero(v0)[0]
        nodes = s0[k]
        rep, jj, srcs = _edge_expand(nodes, degi, s_sorted, indptr)
        slot = k[rep]
        col2 = cb0[slot // P] * 2 + jj
        X0[col2, slot % P] = xhat[srcs] * (S_T * dis2[nodes[rep]])[:, None]
        x0 = _f8(X0.transpose(2, 0, 1).reshape(P, C0P * 2 * P))

        # layer-1 gather indices: real edges only, windowed schedule
        pos0 = np.full(N, -1, np.int64)
        pos0[s0[v0]] = np.nonzero(v0)[0]
        so1 = slot_o[c]
        v1 = so1 >= 0
        flat = np.zeros(Ctot * P, np.int16)
        kk = np.nonzero(v1)[0]
        onodes = so1[kk]
        orep, ojj, osrcs = _edge_expand_noself(onodes, degi, s_sorted,
                                               indptr)
        spos = pos0[osrcs]
        st = spos // P // 4
        w_e = st // WIN_STAGES if NW == 2 else np.zeros(len(spos), np.int64)
        loc = (1 + (st - w_e * WIN_STAGES) * 512 + (spos % P) * 4 +
               (spos // P) % 4)
        oslot = kk[orep]
        tt = oslot // P
        pp = oslot % P
        # rank within (own slot, window)
        o = np.lexsort((ojj, w_e, orep))
        so_rep, so_w = orep[o], w_e[o]
        grp_change = np.ones(len(o), bool)
        grp_change[1:] = (so_rep[1:] != so_rep[:-1]) | \
            (so_w[1:] != so_w[:-1])
        gid = np.cumsum(grp_change) - 1
        starts = np.nonzero(grp_change)[0]
        rank_sorted = np.arange(len(o)) - starts[gid]
        rank = np.empty(len(o), np.int64)
        rank[o] = rank_sorted
        wb = np.asarray(wbase, np.int64)
        cbw = np.stack([np.asarray(scheds[w][2][:T1], np.int64)
                        for w in range(NW)], axis=1)      # [T1, NW]
        col = wb[w_e] + cbw[tt, w_e] + rank
        flat[col * P + pp] = loc.astype(np.int16)
        wrapped = np.tile(flat.reshape(-1, 16).T, (8, 1))

        def _scales(s, v, T, vec):
            iv = np.where(v, vec[np.clip(s, 0, None)], 0).astype(np.float32)
            return iv.reshape(1, T * P)

        inv0 = _scales(s0, v0, T0, dis)      # bias seed layer 0 (non-ZB)
        inv1 = _scales(so1, v1, T1, invd)    # bias seed layer 1 (non-ZB)
        mct = np.zeros((T1 * P, G), np.float32)
        mct[v1] = (Mhat[:, so1[v1]] * (dis[so1[v1]] / S_H2)[None, :]).T
        mct_pm = np.ascontiguousarray(
            mct.reshape(T1, P, G).transpose(1, 0, 2).reshape(P, T1 * G))
        cores.append(dict(x0=x0, idx1=wrapped,
                          inv0=inv0.astype(BF16), inv1=inv1.astype(BF16),
                          mct=mct_pm.astype(BF16), flat1=flat))

    w0d = np.concatenate([np.asarray(W0, np.float32)] * 2, axis=1)
    id1 = np.eye(P, dtype=np.float32)
    shared = dict(
        w0d=_f8(S_W0 * w0d),
        w1=np.ascontiguousarray(W1, np.float32).reshape(2, P, HID
                                                        ).astype(BF16),
        b0r=(S_T * S_W0 * np.ascontiguousarray(b0, np.float32)
             ).reshape(1, HID).astype(BF16),
        b1r=(S_H1 * np.ascontiguousarray(b1, np.float32)
             ).reshape(1, HID).astype(BF16),
        ident2=_f8(np.concatenate([id1, id1], axis=1)),   # [P, 2*P]
    )
    zero_bias = bool(np.all(np.asarray(b0) == 0) and
                     np.all(np.asarray(b1) == 0))
    meta = dict(N=N, IN=IN, HID=HID, G=G, SH=SH, T0=T0, T1=T1, NW=NW,
                C0P=C0P, C1w=tuple(C1w), zero_bias=zero_bias,
                D0P=D0Pt, D1=D1t)
    fin = dict(W2=np.asarray(W2, np.float32), b2=np.asarray(b2, np.float32))
    return meta, shared, cores, fin


# ---------------------------------------------------------------------------
# Pure-numpy emulation of the device program (validation / debugging)
# ---------------------------------------------------------------------------

def _emulate(meta, shared, cores, fin):
    T0, T1, HID, G, NW = (meta[k] for k in ("T0", "T1", "HID", "G", "NW"))
    D0P, D1 = meta["D0P"], meta["D1"]
    _, colbase0 = _schedule0(D0P)
    C0P = colbase0[T0]
    scheds = [_schedule_w(tuple(D1[t][w] for t in range(T1)),
              GCOLS if w == 0 else GCOLS1)
              for w in range(NW)]
    C1w = [s[3] for s in scheds]
    wbase = [0, C1w[0]][:NW]
    w0 = shared["w0d"].astype(np.float32)[:, :HID]
    w1 = shared["w1"].astype(np.float32).reshape(2 * P, HID)
    b0 = shared["b0r"].astype(np.float32)[0]
    b1 = shared["b1r"].astype(np.float32)[0]
    ZB = meta["zero_bias"]

    Y = np.zeros((G, HID), np.float32)
    for cd in cores:
        tab = cd["x0"].astype(np.float32).reshape(P, C0P * 2, P)
        tab = tab.transpose(1, 2, 0)
        inv0 = cd["inv0"].astype(np.float32)[0]
        pre = np.zeros((T0 * P, HID), np.float32)
        if not ZB:
            pre += inv0[:, None] * b0[None, :]
        for t in range(T0):
            for c2 in range(D0P[t] * 2):
                pre[t * P:(t + 1) * P] += tab[colbase0[t] * 2 + c2] @ w0
        h1 = np.clip(np.maximum(pre * ACT0_SCALE, 0), 0, 240
                     ).astype(F8).astype(np.float32)     # [T0*P, HID]

        # windowed h1 storage (loc arithmetic mirrors the device layout)
        h1win = np.zeros((NW, WSTR, HID), np.float32)
        sl = np.arange(T0 * P)
        st = sl // P // 4
        w_s = st // WIN_STAGES
        loc = 1 + (st - w_s * WIN_STAGES) * 512 + (sl % P) * 4 + \
            (sl // P) % 4
        h1win[w_s, loc] = h1

        flat = cd["flat1"]
        inv1 = cd["inv1"].astype(np.float32)[0]
        u_all = np.zeros((T1 * P, 2 * P), np.float32)
        for t in range(T1):
            # phase A: self + window-0 columns, bf16 stash
            uA = h1[t * P:(t + 1) * P].copy()
            cb = scheds[0][2][t]
            for j in range(D1[t][0]):
                col = wbase[0] + cb + j
                uA += h1win[0, flat[col * P:(col + 1) * P]]
            uA = uA.astype(BF16).astype(np.float32)
            if NW == 2 and D1[t][1] > 0:
                uB = np.zeros((P, HID), np.float32)
                cb = scheds[1][2][t]
                for j in range(D1[t][1]):
                    col = wbase[1] + cb + j
                    uB += h1win[1, flat[col * P:(col + 1) * P]]
                uA = (uA + uB).astype(BF16).astype(np.float32)
            u_all[t * P:(t + 1) * P] = uA
        pre1 = u_all @ w1
        if not ZB:
            pre1 += inv1[:, None] * b1[None, :]
        h2 = np.clip(np.maximum(pre1 * ACT1_SCALE, 0), 0, 240
                     ).astype(F8).astype(np.float32)
        mct = cd["mct"].astype(np.float32).reshape(
            P, T1, G).transpose(1, 0, 2).reshape(T1 * P, G)
        Y += mct.T @ h2
    return Y @ fin["W2"] + fin["b2"]


# ---------------------------------------------------------------------------
# Bass device program
# ---------------------------------------------------------------------------

def _build(meta):
    import concourse.bass as bass
    import concourse.mybir as mybir
    import concourse.tile as tile
    from concourse import bacc, library_config
    from concourse.tile_rust import add_dep_helper

    F32, I16 = mybir.dt.float32, mybir.dt.int16
    BF = mybir.dt.bfloat16
    F8D = mybir.dt.float8e4
    RELU = mybir.ActivationFunctionType.Relu
    COPY = mybir.ActivationFunctionType.Copy
    DR = mybir.MatmulPerfMode.DoubleRow

    IN, HID, G = meta["IN"], meta["HID"], meta["G"]
    T0, T1, NW = meta["T0"], meta["T1"], meta["NW"]
    C0P, C1w = meta["C0P"], meta["C1w"]
    D0P, D1 = meta["D0P"], meta["D1"]
    ZB = meta["zero_bias"]
    S0 = T0 // 4
    SOWN = T1 // 4
    groups0, colbase0 = _schedule0(D0P)
    scheds = [_schedule_w(tuple(D1[t][w] for t in range(T1)),
              GCOLS if w == 0 else GCOLS1)
              for w in range(NW)]
    wbase = [0, C1w[0]][:NW]
    Ctot = sum(C1w)
    c2c = [_call_of_col(scheds[w][1]) for w in range(NW)]

    nc = bacc.Bacc("TRN2", target_bir_lowering=False, debug=False,
                   num_devices=NCORES)

    t_x0 = nc.dram_tensor("x0", [P, C0P * 2 * P], F8D, kind="ExternalInput")
    t_idx1 = nc.dram_tensor("idx1", [P, Ctot * 8], I16,
                            kind="ExternalInput")
    t_w0d = nc.dram_tensor("w0d", [IN, 2 * HID], F8D, kind="ExternalInput")
    t_w1 = nc.dram_tensor("w1", [2, P, HID], BF, kind="ExternalInput")
    t_mct = nc.dram_tensor("mct", [P, T1 * G], BF, kind="ExternalInput")
    t_id2 = nc.dram_tensor("ident2", [P, 2 * P], F8D, kind="ExternalInput")
    t_inv0 = nc.dram_tensor("inv0", [1, T0 * P], BF, kind="ExternalInput")
    t_inv1 = nc.dram_tensor("inv1", [1, T1 * P], BF, kind="ExternalInput")
    t_b0 = nc.dram_tensor("b0r", [1, HID], BF, kind="ExternalInput")
    t_b1 = nc.dram_tensor("b1r", [1, HID], BF, kind="ExternalInput")
    t_out = nc.dram_tensor("outp", [P, 2, G], F32, kind="ExternalOutput")

    with tile.TileContext(nc) as tc:
        with (
            tc.tile_pool(name="const", bufs=1) as cpool,
            tc.tile_pool(name="ut", bufs=6) as upool,
            tc.tile_pool(name="stage", bufs=3) as spool,
            tc.tile_pool(name="resid", bufs=1) as rpool,
            tc.tile_pool(name="g1", bufs=3) as gpool,
            tc.tile_pool(name="dram", bufs=1, space="DRAM") as dpool,
        ):
            lib = nc.gpsimd.load_library(library_config.mlp)

            def cload(t, shape, dt):
                s = cpool.tile(shape, dt, tag=t.name)
                nc.sync.dma_start(s[:], t[:])
                return s

            ident2 = cload(t_id2, [P, 2 * P], F8D)
            w0d = cload(t_w0d, [IN, 2 * HID], F8D)
            w1 = cpool.tile([P, 2, HID], BF, tag="w1")
            nc.sync.dma_start(w1[:], t_w1[:].rearrange("j p h -> p j h"))
            if not ZB:
                b0r = cload(t_b0, [1, HID], BF)
                b1r = cload(t_b1, [1, HID], BF)

            h1h = dpool.tile([NW * WSTR, HID], F8D)
            # persistent SBUF: own-tile h1 (fp8), phase-A stash (bf16),
            # h2 (fp8)
            h1sb = rpool.tile([P, T1 * HID], F8D, tag="h1sb")
            stash = rpool.tile([P, T1 * 2 * P], BF, tag="stash")
            h2sb = rpool.tile([P, T1 * HID], F8D, tag="h2sb")

            # early loads on the Activation HWDGE queue
            idx1 = cpool.tile([P, Ctot * 8], I16, tag="idx1")
            nc.scalar.dma_start(idx1[:], t_idx1[:])
            mct_all = cpool.tile([P, T1 * G], BF, tag="mct_all")
            nc.scalar.dma_start(mct_all[:], t_mct[:])

            # h1h_writes[w]: writes a layer-1 gather from window w waits on
            h1h_writes = [[] for _ in range(NW)]
            zt = spool.tile([P, 4 * HID], F8D, tag="zrow")
            nc.vector.memset(zt[:], 0.0)
            for w in range(NW):
                h1h_writes[w].append(nc.scalar.dma_start(
                    h1h[w * WSTR:w * WSTR + 1, :], zt[0:1, :HID]))

            # ---------------- Layer 0 ----------------
            # halo-stage write batches (<=4 stages, never across a window)
            batch_of = {}
            nhalo = S0 - SOWN
            hb = 0
            while hb < nhalo:
                s_ = SOWN + hb
                w_ = s_ // WIN_STAGES
                nb = min(4, nhalo - hb, (w_ + 1) * WIN_STAGES - s_)
                for k_ in range(nb):
                    batch_of[hb + k_] = (hb, nb)
                hb += nb
            hstage = {}
            with tc.tile_pool(name="x0p", bufs=4) as xpool, \
                 tc.tile_pool(name="pre0", bufs=3, space="PSUM") as ppool0:
                pre4 = None
                gi = 0
                xt = None
                off = 0
                for t in range(T0):
                    if gi < len(groups0) and t == groups0[gi][0]:
                        t0g, t1g = groups0[gi]
                        off = colbase0[t0g]
                        ncol = colbase0[t1g] - off
                        xt = xpool.tile([P, GC0 * 2 * P], F8D, tag="x0")
                        nc.sync.dma_start(
                            xt[:, :ncol * 2 * P],
                            t_x0[:, off * 2 * P:(off + ncol) * 2 * P])
                        gi += 1
                    q = t % 4
                    if q == 0:
                        pre4 = ppool0.tile([P, 4, HID], F32,
                                           tag="pre0", space="PSUM")
                    ncp = D0P[t]
                    base = colbase0[t] - off
                    if not ZB:
                        sd = upool.tile([1, P], BF, tag="seed")
                        nc.sync.dma_start(sd[:], t_inv0[:, t * P:(t + 1) * P])
                        nc.tensor.matmul(pre4[:, q, :], lhsT=sd[:],
                                         rhs=b0r[:], start=True, stop=False)
                    for c in range(ncp):
                        nc.tensor.matmul(
                            pre4[:, q, :],
                            lhsT=xt[:, (base + c) * 2 * P:
                                    (base + c + 1) * 2 * P
                                    ].rearrange("p (two m) -> p two m",
                                                two=2),
                            rhs=w0d[:].rearrange("p (two h) -> p two h",
                                                 two=2),
                            start=(ZB and c == 0), stop=(c == ncp - 1),
                            perf_mode=DR)
                    if q == 3:
                        s = t // 4
                        w = s // WIN_STAGES
                        if s < SOWN:
                            dst_sb = h1sb[:, s * 4 * HID:(s + 1) * 4 * HID]
                        else:
                            hb = s - SOWN
                            b0_, nb = batch_of[hb]
                            if b0_ not in hstage:
                                bt_new = spool.tile(
                                    [P, 4 * 4 * HID], F8D, tag="h1stage",
                                    name=f"h1stage_{b0_}")
                                hstage[b0_] = bt_new
                            bt = hstage[b0_]
                            dst_sb = bt[:, (hb - b0_) * 4 * HID:
                                        (hb - b0_ + 1) * 4 * HID]
                        # alternate the ReLU between Activation and DVE so
                        # neither sequencer paces layer 0
                        if s % 2 == 0:
                            nc.scalar.activation(
                                dst_sb.rearrange("p (j h) -> p j h", j=4),
                                pre4[:, :, :], RELU, bias=0.0,
                                scale=ACT0_SCALE)
                        else:
                            nc.vector.tensor_scalar(
                                dst_sb.rearrange("p (j h) -> p j h", j=4),
                                pre4[:, :, :], ACT0_SCALE, 0.0,
                                op0=mybir.AluOpType.mult,
                                op1=mybir.AluOpType.max)
                        if s == SOWN - 1:
                            # one batched write for the whole own region
                            h1h_writes[0].append(nc.scalar.dma_start(
                                h1h[1:1 + SOWN * 512, :].rearrange(
                                    "(s2 p j) h -> p s2 j h", p=P, j=4),
                                h1sb[:].rearrange(
                                    "p (s2 j h) -> p s2 j h",
                                    s2=SOWN, j=4)))
                        elif s >= SOWN:
                            hb = s - SOWN
                            b0_, nb = batch_of[hb]
                            if hb == b0_ + nb - 1:   # batch complete
                                sb0 = SOWN + b0_
                                r0 = (w * WSTR + 1 +
                                      (sb0 - w * WIN_STAGES) * 512)
                                bt = hstage.pop(b0_)
                                h1h_writes[w].append(nc.scalar.dma_start(
                                    h1h[r0:r0 + nb * 512, :].rearrange(
                                        "(s2 p j) h -> p s2 j h",
                                        p=P, j=4),
                                    bt[:, :nb * 4 * HID].rearrange(
                                        "p (s2 j h) -> p s2 j h",
                                        s2=nb, j=4)))

            # ---------------- Layer 1 + fused pool ----------------
            def do_calls(w, ids):
                out = {}
                for ci in ids:
                    ncols, coff = scheds[w][1][ci]
                    goff = wbase[w] + coff
                    gt = gpool.tile([P, CCAP * HID], F8D, tag="g")
                    gi_ = nc.gpsimd.dma_gather(
                        gt[:, :ncols * HID].rearrange(
                            "p (j d) -> p j d", j=ncols),
                        h1h[w * WSTR:(w + 1) * WSTR, :],
                        idx1[:, goff * 8:(goff + ncols) * 8],
                        ncols * P, ncols * P, HID, single_packet=False)
                    add_dep_helper(gi_.ins, lib.ins, True,
                                   "gather after lib")
                    for d in h1h_writes[w]:
                        add_dep_helper(gi_.ins, d.ins, True,
                                       "gather after src")
                    out[ci] = gt
                return out

            def grp_call_ids(w, t0g, t1g):
                ids = set()
                cb = scheds[w][2]
                for tt in range(t0g, t1g):
                    for j in range(D1[tt][w]):
                        ids.add(c2c[w][cb[tt] + j][0])
                return sorted(ids)

            def agg_run(ps, i, bufs, w, t, with_self):
                """Accumulate tile t's window-w columns into
                ps[:, 2*i+fh, :] (i = tile index within the pair)."""
                nd = D1[t][w]
                cb = scheds[w][2][t]
                for fh in range(2):
                    ops = []
                    if with_self:
                        ops.append((h1sb[:, t * HID + fh * P:
                                         t * HID + (fh + 1) * P],
                                    ident2[:, :P], False))
                    j = 0
                    while j < nd:
                        ci, lc = c2c[w][cb + j]
                        gt = bufs[ci]
                        if j + 1 < nd:
                            ci2, lc2 = c2c[w][cb + j + 1]
                            assert ci2 == ci and lc2 == lc + 1
                            lhsT = gt[:, lc * HID:(lc + 2) * HID].rearrange(
                                "p (two fh f) -> p two fh f", two=2, fh=2
                            )[:, :, fh, :]
                            rhs = ident2[:].rearrange(
                                "p (two m) -> p two m", two=2)
                            ops.append((lhsT, rhs, True))
                            j += 2
                        else:
                            lhsT = gt[:, lc * HID + fh * P:
                                      lc * HID + (fh + 1) * P]
                            ops.append((lhsT, ident2[:, :P], False))
                            j += 1
                    for k, (lhsT, rhs, dr) in enumerate(ops):
                        nc.tensor.matmul(
                            ps[:, 2 * i + fh, :], lhsT=lhsT, rhs=rhs,
                            start=(k == 0), stop=(k == len(ops) - 1),
                            perf_mode=DR if dr else None)

            with tc.tile_pool(name="aggps", bufs=3, space="PSUM") as apool, \
                 tc.tile_pool(name="pre1", bufs=3, space="PSUM") as ppool1, \
                 tc.tile_pool(name="outps", bufs=1, space="PSUM") as opool:
                # two separate banks: the fh accumulation chains interleave,
                # and interleaved open chains must not share a PSUM bank
                opsT0 = opool.tile([P, G], F32, tag="outps0", space="PSUM")
                opsT1 = opool.tile([P, G], F32, tag="outps1", space="PSUM")
                opsT = [opsT0[:], opsT1[:]]
                # pooling matmuls lag a few pairs behind the W1/activation
                # pipeline so the in-order PE never stalls on a fresh h2;
                # start/stop flags follow EMISSION order (w1-free pairs are
                # pooled already during phase A)
                pend_pool = []
                pool_n = [0]

                def pool_pair(t0_):
                    for i in range(2):
                        t = t0_ + i
                        for fh in range(2):
                            nc.tensor.matmul(
                                opsT[fh],
                                lhsT=h2sb[:, t * HID + fh * P:
                                          t * HID + (fh + 1) * P],
                                rhs=mct_all[:, t * G:(t + 1) * G],
                                start=(pool_n[0] == 0),
                                stop=(pool_n[0] == T1 - 1))
                        pool_n[0] += 1

                def queue_pool(t0_):
                    pend_pool.append(t0_)
                    while len(pend_pool) > 3:
                        pool_pair(pend_pool.pop(0))

                def finish_pair(t0_, uts):
                    pre1 = ppool1.tile([P, 2, HID], F32, tag="pre1",
                                       space="PSUM")
                    for i, ut in enumerate(uts):
                        t = t0_ + i
                        if not ZB:
                            sd = upool.tile([1, P], BF, tag="seed")
                            nc.sync.dma_start(
                                sd[:], t_inv1[:, t * P:(t + 1) * P])
                            nc.tensor.matmul(pre1[:, i, :], lhsT=sd[:],
                                             rhs=b1r[:], start=True,
                                             stop=False)
                        for fh in range(2):
                            nc.tensor.matmul(
                                pre1[:, i, :],
                                lhsT=ut[:, fh * P:(fh + 1) * P],
                                rhs=w1[:, fh, :],
                                start=(ZB and fh == 0), stop=(fh == 1))
                    nc.scalar.activation(
                        h2sb[:, t0_ * HID:(t0_ + 2) * HID].rearrange(
                            "p (j h) -> p j h", j=2),
                        pre1[:, :, :], RELU, bias=0.0, scale=ACT1_SCALE)
                    queue_pool(t0_)

                def stash_ut(t):
                    return stash[:, t * 2 * P:(t + 1) * 2 * P]

                ADD = mybir.AluOpType.add
                MUL = mybir.AluOpType.mult

                # phase A: self + window-0 columns -> bf16 stash (pairs);
                # pairs with no window-1 columns finish completely here,
                # overlapping the window-1 gathers
                nca = 0
                for (t0g, t1g) in scheds[0][0]:
                    bufs = do_calls(0, grp_call_ids(0, t0g, t1g))
                    for t0_ in range(t0g, t1g, 2):
                        ps = apool.tile([P, 4, P], F32, tag="aggps",
                                        space="PSUM")
                        agg_run(ps, 0, bufs, 0, t0_, True)
                        agg_run(ps, 1, bufs, 0, t0_ + 1, True)
                        dst = stash[:, t0_ * 2 * P:(t0_ + 2) * 2 * P
                                    ].rearrange("p (j f) -> p j f", j=4)
                        # alternate the copy between DVE and Activation
                        if nca % 2 == 0:
                            nc.vector.tensor_copy(dst, ps[:, :, :])
                        else:
                            nc.scalar.activation(dst, ps[:, :, :], COPY,
                                                 bias=0.0, scale=1.0)
                        nca += 1
                        if NW == 1 or (D1[t0_][1] == 0 and
                                       D1[t0_ + 1][1] == 0):
                            finish_pair(t0_, (stash_ut(t0_),
                                              stash_ut(t0_ + 1)))

                # phase B: window-1 columns + stash -> uT, then W1 +
                # activation + pooling per remaining tile pair
                if NW == 2:
                    for (t0g, t1g) in scheds[1][0]:
                        ids = grp_call_ids(1, t0g, t1g)
                        bufs = do_calls(1, ids) if ids else {}
                        for t0_ in range(t0g, t1g, 2):
                            d0, d1 = D1[t0_][1], D1[t0_ + 1][1]
                            if d0 == 0 and d1 == 0:
                                continue  # finished in phase A
                            ps = apool.tile([P, 4, P], F32, tag="aggps",
                                            space="PSUM")
                            if d0 > 0:
                                agg_run(ps, 0, bufs, 1, t0_, False)
                            if d1 > 0:
                                agg_run(ps, 1, bufs, 1, t0_ + 1, False)
                            if d0 > 0 and d1 > 0:
                                up = upool.tile([P, 4 * P], BF, tag="ut")
                                nc.vector.scalar_tensor_tensor(
                                    up[:].rearrange("p (j f) -> p j f",
                                                    j=4),
                                    ps[:, :, :], 1.0,
                                    stash[:, t0_ * 2 * P:
                                          (t0_ + 2) * 2 * P].rearrange(
                                        "p (j f) -> p j f", j=4),
                                    op0=MUL, op1=ADD)
                                uts = (up[:, :2 * P], up[:, 2 * P:])
                            else:
                                i = 0 if d0 > 0 else 1
                                t = t0_ + i
                                up = upool.tile([P, 2 * P], BF, tag="ut")
                                nc.vector.scalar_tensor_tensor(
                                    up[:].rearrange("p (j f) -> p j f",
                                                    j=2),
                                    ps[:, 2 * i:2 * i + 2, :], 1.0,
                                    stash_ut(t).rearrange(
                                        "p (j f) -> p j f", j=2),
                                    op0=MUL, op1=ADD)
                                uts = (up[:], stash_ut(t0_ + 1)) if i == 0 \
                                    else (stash_ut(t0_), up[:])
                            finish_pair(t0_, uts)
                while pend_pool:
                    pool_pair(pend_pool.pop(0))

                osb = spool.tile([P, 2 * G], F32, tag="osb")
                for fh in range(2):
                    nc.vector.tensor_copy(osb[:, fh * G:(fh + 1) * G],
                                          opsT[fh])
                nc.sync.dma_start(t_out[:],
                                  osb[:].rearrange("p (j g) -> p j g", j=2))

    nc.compile()
    return nc


# ---------------------------------------------------------------------------
# Entry point
# ---------------------------------------------------------------------------

_cache = {}


def _get_nc(meta):
    key = hashlib.sha1(repr(sorted(meta.items())).encode()).hexdigest()
    if key not in _cache:
        _cache[key] = _build(meta)
    return _cache[key]


def _in_maps(shared, cores):
    maps = []
    for cd in cores:
        m = dict(shared)
        m.update({k: cd[k] for k in
                  ("x0", "idx1", "inv0", "inv1", "mct")})
        maps.append(m)
    return maps


def _run_device(meta, shared, cores):
    from concourse.bass_utils import run_bass_kernel_spmd
    nc = _get_nc(meta)
    res = run_bass_kernel_spmd(nc, _in_maps(shared, cores),
                               core_ids=list(range(NCORES)))
    return [r["outp"] for r in res.results]


def kernel(**inputs):
    meta, shared, cores, fin = _preprocess(**inputs)
    outs = _run_device(meta, shared, cores)
    YT = np.sum(np.stack(outs), axis=0, dtype=np.float32)  # [P, 2, G]
    G = YT.shape[2]
    Y = YT.transpose(2, 1, 0).reshape(G, 2 * P)            # [G, HID]
    out = Y @ fin["W2"] + fin["b2"]
    return out.astype(np.float32)


def profile_run(meta, shared, cores, trace_cores=None):
    """Profiled exec time in ns: NTFF trace when available, else the
    instruction-cost-model timeline simulation of the compiled program."""
    from concourse.bass_utils import run_bass_kernel_spmd
    nc = _get_nc(meta)
    try:
        res = run_bass_kernel_spmd(nc, _in_maps(shared, cores),
                                   core_ids=list(range(NCORES)), trace=True,
                                   trace_cores=trace_cores)
        if res.exec_time_ns is not None:
            print("profile:", res.instructions_and_trace[1]
                  if res.instructions_and_trace else None)
            return res.exec_time_ns
    except Exception as e:
        print(f"NTFF trace unavailable ({type(e).__name__}); "
              "using cost-model timeline")
    from concourse.timeline_sim import TimelineSim
    ts = TimelineSim(nc, trace=False)
    ts.simulate()
    return int(ts.time)
